# revision 1
# baseline (speedup 1.0000x reference)
"""Trainium2 Bass kernel: single-head causal self-attention.

Reference computation (per batch b):
    Q = x @ Wq ; K = x @ Wk ; V = x @ Wv          (x: [S, D])
    S_sc = Q @ K^T / sqrt(D), causal masked
    out  = softmax(S_sc) @ V

Sharding: 8 cores, 4 batches -> core c handles batch b = c//2 and query
half h = c%2 (1024 query rows), with full K/V for that batch computed
on-core (redundantly for the pair). Uniform SPMD program; per-core
behavior comes only from input data (xqT slice + global-q-index vector
used to build the causal mask on device).

Layout strategy (all fp32):
  - Host passes x[b]^T so the contraction dim (d_in) lands on partitions.
  - K^T [d, S] stays resident in SBUF; V [S, d] is staged to DRAM during
    the projection phase and streamed back per q-strip.
  - Scores are computed TRANSPOSED: S^T[k, q] = sum_d K^T[d,k] * Q^T[d,q],
    so softmax's k-reduction lands on the partition dim; the sum is done
    with an extra N=1 matmul against a ones vector (riding the same
    stationary P^T tile as the P@V matmuls), avoiding any P transposes.
  - No max-subtraction in softmax: scores ~ N(0,1), exp is safe in fp32.
  - Causal mask built on device: mask[k,q] = (q_global >= k_global),
    multiplied into exp(S^T) (multiplicative 0/1 mask after exp).
"""

import sys

try:
    import concourse.bass as bass  # noqa: F401
except ImportError:
    sys.path.insert(0, "/opt/trn_rl_repo")

import numpy as np

import concourse.bass as bass
import concourse.tile as tile
from concourse import bacc, mybir
from concourse.bass_utils import run_bass_kernel_spmd

B, S, D = 4, 2048, 1024
NQ = 1024  # query rows per core
NK = 2048  # keys per core
P = 128
DT = D // P  # 8 d tiles
KT = NK // P  # 16 k tiles
W = 256  # q-strip width
NSTRIP = NQ // W  # 4 strips
F32 = mybir.dt.float32
SCALE = 1.0 / np.sqrt(np.float32(D))  # 0.03125

_NC_CACHE = {}


def build_nc(mm_dt=F32):
    nc = bacc.Bacc(None, target_bir_lowering=False)
    xkvT = nc.dram_tensor("xkvT", [D, NK], mm_dt, kind="ExternalInput")
    xqT = nc.dram_tensor("xqT", [D, NQ], mm_dt, kind="ExternalInput")
    qg = nc.dram_tensor("qg", [NQ], F32, kind="ExternalInput")
    wq_d = nc.dram_tensor("Wq", [D, D], mm_dt, kind="ExternalInput")
    wk_d = nc.dram_tensor("Wk", [D, D], mm_dt, kind="ExternalInput")
    wv_d = nc.dram_tensor("Wv", [D, D], mm_dt, kind="ExternalInput")
    out_d = nc.dram_tensor("out", [NQ, D], F32, kind="ExternalOutput")
    vdram = nc.dram_tensor("vscratch", [NK, D], mm_dt)

    with tile.TileContext(nc) as tc:
        with (
            tc.tile_pool(name="persist", bufs=1) as persist,
            tc.tile_pool(name="misc", bufs=1) as misc,
        ):
            # Persistent K^T [d, NK] (8 partition-tiles)
            kT = persist.tile([P, DT, NK], mm_dt, tag="kT")

            # Small constants: ones columns (2 wide: fp32r matmuls need an
            # even moving dim), k-index vectors for the mask
            ones_f = misc.tile([P, 2], F32, tag="ones_f")
            nc.vector.memset(ones_f, 1.0)
            ones = misc.tile([P, 2], mm_dt, tag="ones")
            nc.vector.tensor_copy(ones, ones_f)
            pvec_i = misc.tile([P, 1], mybir.dt.int32, tag="pvec_i")
            nc.gpsimd.iota(pvec_i, pattern=[[0, 1]], base=0, channel_multiplier=1)
            pvec = misc.tile([P, 1], F32, tag="pvec")
            nc.vector.tensor_copy(pvec, pvec_i)
            kvecf = misc.tile([P, KT], F32, tag="kvecf")
            for kt in range(KT):
                nc.vector.tensor_scalar_add(kvecf[:, kt : kt + 1], pvec, float(kt * P))

            # ------------- Phase 1: K^T (SBUF) and V (-> DRAM) -------------
            with (
                tc.tile_pool(name="wkv", bufs=1) as wkvp,
                tc.tile_pool(name="xin", bufs=2) as xinp,
                tc.tile_pool(name="vstage", bufs=2) as vsp,
                tc.tile_pool(name="ps1", bufs=4, space="PSUM") as ps1,
            ):
                wk = wkvp.tile([P, DT, D], mm_dt, tag="wk")
                wv = wkvp.tile([P, DT, D], mm_dt, tag="wv")
                wk_t = wk_d.rearrange("(a p) o -> p a o", p=P)
                wv_t = wv_d.rearrange("(a p) o -> p a o", p=P)
                # all of wk first: the first K^T group accumulates over all
                # 8 d_in tiles, so wk's arrival gates PE start; wv is not
                # needed until the V section
                for di in range(DT):
                    nc.scalar.dma_start(wk[:, di, :], wk_t[:, di, :])
                for di in range(DT):
                    nc.scalar.dma_start(wv[:, di, :], wv_t[:, di, :])
                xkvT_t = xkvT.rearrange("(a p) s -> p a s", p=P)

                for qr in range(4):  # quarters of the key sequence
                    s0 = qr * 512
                    xin = xinp.tile([P, DT, 512], mm_dt, tag="xin")
                    nc.sync.dma_start(xin, xkvT_t[:, :, s0 : s0 + 512])
                    # K^T tiles: out[d_out, s] accumulated over d_in
                    for do in range(DT):
                        ps = ps1.tile([P, 512], F32, tag="ps1")
                        for di in range(DT):
                            nc.tensor.matmul(
                                ps,
                                wk[:, di, do * P : (do + 1) * P],
                                xin[:, di, :],
                                start=(di == 0),
                                stop=(di == DT - 1),
                            )
                        nc.vector.tensor_copy(kT[:, do, s0 : s0 + 512], ps)
                    # V tiles: out[s, d_out] accumulated over d_in -> DRAM
                    for st in range(4):
                        gst = qr * 4 + st
                        vstage = vsp.tile([P, D], mm_dt, tag="vstage")
                        for dh in range(2):
                            ps = ps1.tile([P, 512], F32, tag="ps1")
                            for di in range(DT):
                                nc.tensor.matmul(
                                    ps,
                                    xin[:, di, st * P : (st + 1) * P],
                                    wv[:, di, dh * 512 : (dh + 1) * 512],
                                    start=(di == 0),
                                    stop=(di == DT - 1),
                                )
                            nc.vector.tensor_copy(vstage[:, dh * 512 : (dh + 1) * 512], ps)
                        nc.sync.dma_start(vdram[gst * P : (gst + 1) * P, :], vstage)

            # ---------------- Phase 2: per-q-strip attention ----------------
            with (
                tc.tile_pool(name="wqp", bufs=1) as wqp,
                tc.tile_pool(name="strip", bufs=1) as strip,
                tc.tile_pool(name="vs2", bufs=4) as vs2,
                tc.tile_pool(name="sm", bufs=4) as sm,
                tc.tile_pool(name="outp", bufs=2) as outp,
                tc.tile_pool(name="ps2", bufs=2, space="PSUM") as ps2p,
                tc.tile_pool(name="psc", bufs=2, space="PSUM") as pscp,
                tc.tile_pool(name="psl", bufs=2, space="PSUM") as pslp,
            ):
                wq = wqp.tile([P, DT, D], mm_dt, tag="wq")
                wq_t = wq_d.rearrange("(a p) o -> p a o", p=P)
                for di in range(DT):
                    nc.scalar.dma_start(wq[:, di, :], wq_t[:, di, :])
                xqT_t = xqT.rearrange("(a p) s -> p a s", p=P)

                for qs in range(NSTRIP):
                    q0 = qs * W
                    qx = strip.tile([P, DT, W], mm_dt, tag="qx", bufs=2)
                    nc.scalar.dma_start(qx, xqT_t[:, :, q0 : q0 + W])
                    # Q^T strip [d, W]
                    qT = strip.tile([P, DT, W], mm_dt, tag="qT")
                    for do in range(DT):
                        ps = ps2p.tile([P, W], F32, tag="ps2")
                        for di in range(DT):
                            nc.tensor.matmul(
                                ps,
                                wq[:, di, do * P : (do + 1) * P],
                                qx[:, di, :],
                                start=(di == 0),
                                stop=(di == DT - 1),
                            )
                        nc.vector.tensor_copy(qT[:, do, :], ps)

                    # broadcast global q indices for this strip to all partitions
                    qgrid = sm.tile([P, W], F32, tag="qgrid")
                    qg_sl = qg[q0 : q0 + W]
                    nc.gpsimd.dma_start(
                        qgrid,
                        bass.AP(
                            tensor=qg_sl.tensor,
                            offset=qg_sl.offset,
                            ap=[[0, P]] + list(qg_sl.ap),
                        ),
                    )

                    # S^T strip -> exp -> mask -> P^T strip.
                    # Causal interleave: this strip holds global q-tiles
                    # 2j+h for j in {2qs, 2qs+1}, so k-tiles >= ext_kt are
                    # fully masked and skipped at compile time.
                    ext_kt = 4 * (qs + 1)
                    pT = strip.tile([P, KT, W], mm_dt, tag="pT")
                    for kt in range(ext_kt):
                        ps = ps2p.tile([P, W], F32, tag="ps2")
                        for di in range(DT):
                            nc.tensor.matmul(
                                ps,
                                kT[:, di, kt * P : (kt + 1) * P],
                                qT[:, di, :],
                                start=(di == 0),
                                stop=(di == DT - 1),
                            )
                        et = sm.tile([P, W], F32, tag="et")
                        nc.scalar.activation(
                            et, ps, mybir.ActivationFunctionType.Exp, scale=float(SCALE)
                        )
                        mt = sm.tile([P, W], F32, tag="mt")
                        nc.vector.tensor_scalar(
                            mt,
                            qgrid,
                            kvecf[:, kt : kt + 1],
                            None,
                            op0=mybir.AluOpType.is_ge,
                        )
                        nc.vector.tensor_mul(pT[:, kt, :], et, mt)

                    # context = P^T.T @ V (V streamed from DRAM, kt-outer),
                    # row-sums l via ones column riding the same stationary P^T
                    ncq = W // P
                    cps = [
                        pscp.tile([P, D], F32, tag="psc", name=f"cps{i}")
                        for i in range(ncq)
                    ]
                    lps = [
                        pslp.tile([P, 2], F32, tag="psl", name=f"lps{i}")
                        for i in range(ncq)
                    ]
                    for kt in range(ext_kt):
                        vt_t = vs2.tile([P, D], mm_dt, tag="vstrip")
                        nc.sync.dma_start(vt_t, vdram[kt * P : (kt + 1) * P, :])
                        for qt in range(ncq):
                            ej = 2 * (qs * ncq + qt) + 2  # this position's extent
                            if kt >= ej:
                                continue
                            lhs = pT[:, kt, qt * P : (qt + 1) * P]
                            nc.tensor.matmul(
                                cps[qt][:, 0:512],
                                lhs,
                                vt_t[:, 0:512],
                                start=(kt == 0),
                                stop=(kt == ej - 1),
                            )
                            nc.tensor.matmul(
                                cps[qt][:, 512:1024],
                                lhs,
                                vt_t[:, 512:1024],
                                start=(kt == 0),
                                stop=(kt == ej - 1),
                            )
                            nc.tensor.matmul(
                                lps[qt],
                                lhs,
                                ones,
                                start=(kt == 0),
                                stop=(kt == ej - 1),
                            )
                    for qt in range(ncq):
                        qrow = q0 + qt * P
                        rt = sm.tile([P, 1], F32, tag="rt")
                        nc.vector.reciprocal(rt, lps[qt][:, 0:1])
                        ot = outp.tile([P, D], F32, tag="ot")
                        nc.vector.tensor_scalar_mul(ot, cps[qt], rt)
                        nc.sync.dma_start(out_d[qrow : qrow + P, :], ot)
    nc.compile()
    return nc


def _get_nc(key="f32"):
    if key not in _NC_CACHE:
        _NC_CACHE[key] = build_nc(F32 if key == "f32" else mybir.dt.float32r)
    return _NC_CACHE[key]


def _qsel(h):
    """Query rows for core-half h: global q-tiles h, 2+h, ..., 14+h.

    Position j's tile 2j+h needs only k < (2j+h+1)*128, letting the kernel
    skip fully-masked k-tiles at compile time with a core-uniform program."""
    tiles = np.arange(8) * 2 + h
    return (tiles[:, None] * P + np.arange(P)[None, :]).reshape(-1)


def make_in_maps(x, Wq, Wk, Wv):
    x = np.asarray(x, dtype=np.float32)
    Wq = np.ascontiguousarray(np.asarray(Wq, dtype=np.float32))
    Wk = np.ascontiguousarray(np.asarray(Wk, dtype=np.float32))
    Wv = np.ascontiguousarray(np.asarray(Wv, dtype=np.float32))
    in_maps = []
    for c in range(8):
        b, h = c // 2, c % 2
        qsel = _qsel(h)
        in_maps.append(
            {
                "xkvT": np.ascontiguousarray(x[b].T),
                "xqT": np.ascontiguousarray(x[b][qsel].T),
                "qg": qsel.astype(np.float32),
                "Wq": Wq,
                "Wk": Wk,
                "Wv": Wv,
            }
        )
    return in_maps


def kernel(x, Wq, Wk, Wv, _trace=False, _nc_key="f32r"):
    nc = _get_nc(_nc_key)
    in_maps = make_in_maps(x, Wq, Wk, Wv)
    res = run_bass_kernel_spmd(nc, in_maps, core_ids=list(range(8)), trace=_trace)
    out = np.empty((B, S, D), dtype=np.float32)
    for c in range(8):
        b, h = c // 2, c % 2
        out[b, _qsel(h), :] = res.results[c]["out"]
    if _trace:
        kernel.last_results = res
    return out



# revision 6
# speedup vs baseline: 1.2789x; 1.2789x over previous
"""Trainium2 Bass kernel: single-head causal self-attention (fused streaming).

Reference computation (per batch b):
    Q = x @ Wq ; K = x @ Wk ; V = x @ Wv          (x: [S, D])
    S_sc = Q @ K^T / sqrt(D), causal masked
    out  = softmax(S_sc) @ V

Sharding: 8 cores, 4 batches -> core c handles batch b = c//2 and the
interleaved query half h = c%2 (q-tiles 2p+h, 1024 query rows), with full
K/V for that batch computed on-core (redundantly for the pair). Uniform
SPMD program; per-core behavior comes only from input data.

Fused streaming structure (vs the old two-phase kernel): one loop over 4
key-quarters (512 keys each). Quarter qr projects K^T/V/Q^T for those
keys/queries (bf16 storage, fp32 PSUM accumulate), then immediately runs
the attention strip whose 256 queries live in that quarter (their causal
extent is exactly the quarters processed so far). This removes the old
phase-1/phase-2 boundary stall and keeps V resident in SBUF (no DRAM
round-trip / re-streaming).

All matmul operands are bf16 (inputs cast on host): same PE rate as
fp32r at these tile sizes, half the DMA traffic and SBUF footprint.
Scores are computed transposed (S^T[k, q]) so the softmax k-reduction
lands on partitions; row sums ride an extra ones-column matmul on the
same stationary P^T tiles. No max-subtraction (scores ~ N(0,1); exp is
safe in fp32). Causal mask built on device from a global-q-index input.
"""

import sys

try:
    import concourse.bass as bass  # noqa: F401
except ImportError:
    sys.path.insert(0, "/opt/trn_rl_repo")

import ml_dtypes
import numpy as np

import concourse.bass as bass
import concourse.tile as tile
from concourse import bacc, mybir
from concourse.bass_utils import run_bass_kernel_spmd

B, S, D = 4, 2048, 1024
NQ = 1024  # query rows per core
NK = 2048  # keys per core
P = 128
DT = D // P  # 8 d tiles
KT = NK // P  # 16 k tiles
W = 256  # queries per quarter-strip
NQR = 4  # key quarters (512 keys each)
F32 = mybir.dt.float32
BF16 = mybir.dt.bfloat16
SCALE = 1.0 / np.sqrt(np.float32(D))  # 0.03125
BF_NP = ml_dtypes.bfloat16

_NC_CACHE = {}


def build_nc():
    nc = bacc.Bacc(None, target_bir_lowering=False)
    xkvT = nc.dram_tensor("xkvT", [D, NK], BF16, kind="ExternalInput")
    xqT = nc.dram_tensor("xqT", [D, NQ], BF16, kind="ExternalInput")
    qg = nc.dram_tensor("qg", [NQ], F32, kind="ExternalInput")
    wq_d = nc.dram_tensor("Wq", [D, D], BF16, kind="ExternalInput")
    wk_d = nc.dram_tensor("Wk", [D, D], BF16, kind="ExternalInput")
    wv_d = nc.dram_tensor("Wv", [D, D], BF16, kind="ExternalInput")
    out_d = nc.dram_tensor("out", [NQ, D], F32, kind="ExternalOutput")

    with tile.TileContext(nc) as tc:
        with (
            tc.tile_pool(name="persist", bufs=1) as persist,
            tc.tile_pool(name="misc", bufs=1) as misc,
        ):
            # Persistent SBUF residents (bf16): K^T [d, NK], V [k, D] per
            # k-tile, Q^T [d, NQ]
            kT = persist.tile([P, DT, NK], BF16, tag="kT")
            vT = persist.tile([P, KT, D], BF16, tag="vT")
            qT = persist.tile([P, DT, NQ], BF16, tag="qT")
            wk = persist.tile([P, DT, D], BF16, tag="wk")
            wv = persist.tile([P, DT, D], BF16, tag="wv")
            wq = persist.tile([P, DT, D], BF16, tag="wq")

            # Small constants: ones columns (moving operand of the row-sum
            # matmul), k-index vectors for the causal mask
            ones_f = misc.tile([P, 2], F32, tag="ones_f")
            nc.vector.memset(ones_f, 1.0)
            ones = misc.tile([P, 2], BF16, tag="ones")
            nc.vector.tensor_copy(ones, ones_f)
            pvec_i = misc.tile([P, 1], mybir.dt.int32, tag="pvec_i")
            nc.gpsimd.iota(pvec_i, pattern=[[0, 1]], base=0, channel_multiplier=1)
            pvec = misc.tile([P, 1], F32, tag="pvec")
            nc.vector.tensor_copy(pvec, pvec_i)
            kvecf = misc.tile([P, KT], F32, tag="kvecf")
            for kt in range(KT):
                nc.vector.tensor_scalar_add(kvecf[:, kt : kt + 1], pvec, float(kt * P))

            # Weight DMAs (scalar queue; xin/xq ride the sync queue in
            # parallel). wk first: quarter 0's K matmuls gate PE start.
            wk_t = wk_d.rearrange("(a p) o -> p a o", p=P)
            wv_t = wv_d.rearrange("(a p) o -> p a o", p=P)
            wq_t = wq_d.rearrange("(a p) o -> p a o", p=P)
            for di in range(DT):
                nc.scalar.dma_start(wk[:, di, :], wk_t[:, di, :])
            for di in range(DT):
                nc.scalar.dma_start(wv[:, di, :], wv_t[:, di, :])
            for di in range(DT):
                nc.scalar.dma_start(wq[:, di, :], wq_t[:, di, :])

            xkvT_t = xkvT.rearrange("(a p) s -> p a s", p=P)
            xqT_t = xqT.rearrange("(a p) s -> p a s", p=P)

            with (
                tc.tile_pool(name="xin", bufs=2) as xinp,
                tc.tile_pool(name="xq", bufs=2) as xqp,
                tc.tile_pool(name="sm", bufs=4) as sm,
                tc.tile_pool(name="pt", bufs=1) as ptp,
                tc.tile_pool(name="outp", bufs=2) as outp,
                tc.tile_pool(name="psA", bufs=2, space="PSUM") as psA,
                tc.tile_pool(name="psc", bufs=2, space="PSUM") as pscp,
                tc.tile_pool(name="psl", bufs=2, space="PSUM") as pslp,
            ):
                for qr in range(NQR):
                    s0 = qr * 512
                    q0 = qr * W
                    # ---- input slices for this quarter (per-di DMAs for
                    # fine-grained matmul start) ----
                    xin = xinp.tile([P, DT, 512], BF16, tag="xin")
                    for di in range(DT):
                        nc.sync.dma_start(xin[:, di, :], xkvT_t[:, di, s0 : s0 + 512])
                    xq = xqp.tile([P, DT, W], BF16, tag="xq")
                    nc.sync.dma_start(xq, xqT_t[:, :, q0 : q0 + W])

                    # ---- K^T tiles: out[d_out, k] accumulated over d_in ----
                    for do in range(DT):
                        ps = psA.tile([P, 512], F32, tag="psA")
                        for di in range(DT):
                            nc.tensor.matmul(
                                ps,
                                wk[:, di, do * P : (do + 1) * P],
                                xin[:, di, :],
                                start=(di == 0),
                                stop=(di == DT - 1),
                            )
                        nc.vector.tensor_copy(kT[:, do, s0 : s0 + 512], ps)

                    # ---- V tiles: out[k, d_out] accumulated over d_in ----
                    for st in range(4):
                        gkt = qr * 4 + st
                        for dh in range(2):
                            ps = psA.tile([P, 512], F32, tag="psA")
                            for di in range(DT):
                                nc.tensor.matmul(
                                    ps,
                                    xin[:, di, st * P : (st + 1) * P],
                                    wv[:, di, dh * 512 : (dh + 1) * 512],
                                    start=(di == 0),
                                    stop=(di == DT - 1),
                                )
                            nc.vector.tensor_copy(
                                vT[:, gkt, dh * 512 : (dh + 1) * 512], ps
                            )

                    # ---- Q^T strip: out[d_out, q] accumulated over d_in ----
                    for do in range(DT):
                        ps = psA.tile([P, W], F32, tag="psA")
                        for di in range(DT):
                            nc.tensor.matmul(
                                ps,
                                wq[:, di, do * P : (do + 1) * P],
                                xq[:, di, :],
                                start=(di == 0),
                                stop=(di == DT - 1),
                            )
                        nc.vector.tensor_copy(qT[:, do, q0 : q0 + W], ps)

                    # broadcast global q indices for this strip to all
                    # partitions (for the causal mask)
                    qgrid = sm.tile([P, W], F32, tag="qgrid")
                    qg_sl = qg[q0 : q0 + W]
                    nc.gpsimd.dma_start(
                        qgrid,
                        bass.AP(
                            tensor=qg_sl.tensor,
                            offset=qg_sl.offset,
                            ap=[[0, P]] + list(qg_sl.ap),
                        ),
                    )

                    # ---- attention strip qr: S^T -> exp -> mask -> P^T ----
                    # This strip holds global q-tiles 4qr+h and 4qr+2+h, so
                    # k-tiles >= ext_kt are fully masked and skipped.
                    ext_kt = 4 * (qr + 1)
                    pT = ptp.tile([P, KT, W], BF16, tag="pT")
                    for kt in range(ext_kt):
                        ps = psA.tile([P, W], F32, tag="psA")
                        for di in range(DT):
                            nc.tensor.matmul(
                                ps,
                                kT[:, di, kt * P : (kt + 1) * P],
                                qT[:, di, q0 : q0 + W],
                                start=(di == 0),
                                stop=(di == DT - 1),
                            )
                        et = sm.tile([P, W], F32, tag="et")
                        nc.scalar.activation(
                            et, ps, mybir.ActivationFunctionType.Exp, scale=float(SCALE)
                        )
                        mt = sm.tile([P, W], F32, tag="mt")
                        nc.vector.tensor_scalar(
                            mt,
                            qgrid,
                            kvecf[:, kt : kt + 1],
                            None,
                            op0=mybir.AluOpType.is_ge,
                        )
                        nc.vector.tensor_mul(pT[:, kt, :], et, mt)

                    # ---- context = P^T.T @ V (V resident in SBUF) ----
                    ncq = W // P
                    cps = [
                        pscp.tile([P, D], F32, tag="psc", name=f"cps{i}")
                        for i in range(ncq)
                    ]
                    lps = [
                        pslp.tile([P, 2], F32, tag="psl", name=f"lps{i}")
                        for i in range(ncq)
                    ]
                    for kt in range(ext_kt):
                        for qt in range(ncq):
                            ej = 2 * (qr * ncq + qt) + 2  # this position's extent
                            if kt >= ej:
                                continue
                            lhs = pT[:, kt, qt * P : (qt + 1) * P]
                            nc.tensor.matmul(
                                cps[qt][:, 0:512],
                                lhs,
                                vT[:, kt, 0:512],
                                start=(kt == 0),
                                stop=(kt == ej - 1),
                            )
                            nc.tensor.matmul(
                                cps[qt][:, 512:1024],
                                lhs,
                                vT[:, kt, 512:1024],
                                start=(kt == 0),
                                stop=(kt == ej - 1),
                            )
                            nc.tensor.matmul(
                                lps[qt],
                                lhs,
                                ones,
                                start=(kt == 0),
                                stop=(kt == ej - 1),
                            )
                    for qt in range(ncq):
                        qrow = q0 + qt * P
                        rt = sm.tile([P, 1], F32, tag="rt")
                        nc.vector.reciprocal(rt, lps[qt][:, 0:1])
                        ot = outp.tile([P, D], F32, tag="ot")
                        nc.vector.tensor_scalar_mul(ot, cps[qt], rt)
                        nc.sync.dma_start(out_d[qrow : qrow + P, :], ot)
    nc.compile()
    return nc


def _get_nc(key="bf16"):
    if "nc" not in _NC_CACHE:
        _NC_CACHE["nc"] = build_nc()
    return _NC_CACHE["nc"]


def _qsel(h):
    """Query rows for core-half h: global q-tiles h, 2+h, ..., 14+h.

    Position p's tile 2p+h needs only k < (2p+h+1)*128, letting the kernel
    skip fully-masked k-tiles at compile time with a core-uniform program."""
    tiles = np.arange(8) * 2 + h
    return (tiles[:, None] * P + np.arange(P)[None, :]).reshape(-1)


def make_in_maps(x, Wq, Wk, Wv):
    x = np.asarray(x, dtype=np.float32)
    Wq = np.ascontiguousarray(np.asarray(Wq, dtype=np.float32)).astype(BF_NP)
    Wk = np.ascontiguousarray(np.asarray(Wk, dtype=np.float32)).astype(BF_NP)
    Wv = np.ascontiguousarray(np.asarray(Wv, dtype=np.float32)).astype(BF_NP)
    in_maps = []
    for c in range(8):
        b, h = c // 2, c % 2
        qsel = _qsel(h)
        xbT = np.ascontiguousarray(x[b].T).astype(BF_NP)
        in_maps.append(
            {
                "xkvT": xbT,
                "xqT": np.ascontiguousarray(xbT[:, qsel]),
                "qg": qsel.astype(np.float32),
                "Wq": Wq,
                "Wk": Wk,
                "Wv": Wv,
            }
        )
    return in_maps


def kernel(x, Wq, Wk, Wv, _trace=False, _nc_key="bf16"):
    nc = _get_nc(_nc_key)
    in_maps = make_in_maps(x, Wq, Wk, Wv)
    res = run_bass_kernel_spmd(nc, in_maps, core_ids=list(range(8)), trace=_trace)
    out = np.empty((B, S, D), dtype=np.float32)
    for c in range(8):
        b, h = c // 2, c % 2
        out[b, _qsel(h), :] = res.results[c]["out"]
    if _trace:
        kernel.last_results = res
    return out


# revision 7
# speedup vs baseline: 1.4021x; 1.0964x over previous
"""Trainium2 Bass kernel: single-head causal self-attention (fused streaming,
pair-split K/V projection with AllGather exchange).

Reference computation (per batch b):
    Q = x @ Wq ; K = x @ Wk ; V = x @ Wv          (x: [S, D])
    S_sc = Q @ K^T / sqrt(D), causal masked
    out  = softmax(S_sc) @ V

Sharding: 8 cores, 4 batches -> core c handles batch b = c//2 and the
interleaved query half h = c%2 (q-tiles 2p+h, 1024 query rows). The K/V
projections for batch b are split across the pair by output column: core h
computes K^T/V for d_out columns [h*512, (h+1)*512) only (its Wk/Wv input
is the corresponding half of the weight matrix), and the halves are
exchanged with a per-quarter pairwise AllGather through DRAM bounce
buffers. This halves the projection FLOPs vs computing K/V redundantly.

Fused streaming structure: one loop over 4 key-quarters (512 keys each).
Quarter qr projects its K^T/V half + this core's Q^T strip, launches the
AllGather, then runs the attention strip whose 256 queries live in that
quarter (their causal extent is exactly the quarters processed so far).
K^T, V and Q^T all stay resident in SBUF in bf16.

Scores are computed transposed (S^T[k, q]) so the softmax k-reduction
lands on partitions; row sums ride an extra ones-column matmul on the
same stationary P^T tiles. No max-subtraction (scores ~ N(0,1); exp is
safe in fp32). Causal mask built on device from a global-q-index input.
"""

import sys

try:
    import concourse.bass as bass  # noqa: F401
except ImportError:
    sys.path.insert(0, "/opt/trn_rl_repo")

import ml_dtypes
import numpy as np

import concourse.bass as bass
import concourse.tile as tile
from concourse import bacc, mybir
from concourse.bass_utils import run_bass_kernel_spmd

B, S, D = 4, 2048, 1024
NQ = 1024  # query rows per core
NK = 2048  # keys per core
P = 128
DT = D // P  # 8 d tiles
KT = NK // P  # 16 k tiles
W = 256  # queries per quarter-strip
NQR = 4  # key quarters (512 keys each)
HD = D // 2  # per-core K/V projection half (d_out columns)
HDT = DT // 2  # 4 d_out tiles per half
F32 = mybir.dt.float32
BF16 = mybir.dt.bfloat16
SCALE = 1.0 / np.sqrt(np.float32(D))  # 0.03125
BF_NP = ml_dtypes.bfloat16

_NC_CACHE = {}


def build_nc(n_cores=8):
    groups = [[2 * i, 2 * i + 1] for i in range(n_cores // 2)]
    nc = bacc.Bacc(None, target_bir_lowering=False, num_devices=n_cores)
    xkvT = nc.dram_tensor("xkvT", [D, NK], BF16, kind="ExternalInput")
    xqT = nc.dram_tensor("xqT", [D, NQ], BF16, kind="ExternalInput")
    qg = nc.dram_tensor("qg", [NQ], F32, kind="ExternalInput")
    wq_d = nc.dram_tensor("Wq", [D, D], BF16, kind="ExternalInput")
    wk_d = nc.dram_tensor("Wk", [D, HD], BF16, kind="ExternalInput")
    wv_d = nc.dram_tensor("Wv", [D, HD], BF16, kind="ExternalInput")
    out_d = nc.dram_tensor("out", [NQ, D], F32, kind="ExternalOutput")

    with tile.TileContext(nc) as tc:
        with (
            tc.tile_pool(name="persist", bufs=1) as persist,
            tc.tile_pool(name="misc", bufs=1) as misc,
        ):
            # Persistent SBUF residents (bf16): K^T [d, NK], V [k, D] per
            # k-tile, Q^T [d, NQ]
            kT = persist.tile([P, DT, NK], BF16, tag="kT")
            vT = persist.tile([P, KT, D], BF16, tag="vT")
            qT = persist.tile([P, DT, NQ], BF16, tag="qT")
            wk = persist.tile([P, DT, HD], BF16, tag="wk")
            wv = persist.tile([P, DT, HD], BF16, tag="wv")
            wq = persist.tile([P, DT, D], BF16, tag="wq")

            # Small constants: ones columns (moving operand of the row-sum
            # matmul), k-index vectors for the causal mask
            ones_f = misc.tile([P, 2], F32, tag="ones_f")
            nc.vector.memset(ones_f, 1.0)
            ones = misc.tile([P, 2], BF16, tag="ones")
            nc.vector.tensor_copy(ones, ones_f)
            pvec_i = misc.tile([P, 1], mybir.dt.int32, tag="pvec_i")
            nc.gpsimd.iota(pvec_i, pattern=[[0, 1]], base=0, channel_multiplier=1)
            pvec = misc.tile([P, 1], F32, tag="pvec")
            nc.vector.tensor_copy(pvec, pvec_i)
            kvecf = misc.tile([P, KT], F32, tag="kvecf")
            for kt in range(KT):
                nc.vector.tensor_scalar_add(kvecf[:, kt : kt + 1], pvec, float(kt * P))

            # Weight DMAs (scalar queue; xin/xq ride the sync queue in
            # parallel). wk first: quarter 0's K matmuls gate PE start.
            wk_t = wk_d.rearrange("(a p) o -> p a o", p=P)
            wv_t = wv_d.rearrange("(a p) o -> p a o", p=P)
            wq_t = wq_d.rearrange("(a p) o -> p a o", p=P)
            for di in range(DT):
                nc.scalar.dma_start(wk[:, di, :], wk_t[:, di, :])
            for di in range(DT):
                nc.scalar.dma_start(wv[:, di, :], wv_t[:, di, :])
            for di in range(DT):
                nc.scalar.dma_start(wq[:, di, :], wq_t[:, di, :])

            xkvT_t = xkvT.rearrange("(a p) s -> p a s", p=P)
            xqT_t = xqT.rearrange("(a p) s -> p a s", p=P)

            with (
                tc.tile_pool(name="xin", bufs=2) as xinp,
                tc.tile_pool(name="xq", bufs=2) as xqp,
                tc.tile_pool(name="stg", bufs=2) as stgp,
                tc.tile_pool(name="dram", bufs=2, space="DRAM") as dram,
                tc.tile_pool(name="sm", bufs=4) as sm,
                tc.tile_pool(name="pt", bufs=1) as ptp,
                tc.tile_pool(name="outp", bufs=2) as outp,
                tc.tile_pool(name="psA", bufs=2, space="PSUM") as psA,
                tc.tile_pool(name="psc", bufs=2, space="PSUM") as pscp,
                tc.tile_pool(name="psl", bufs=2, space="PSUM") as pslp,
            ):
                for qr in range(NQR):
                    s0 = qr * 512
                    q0 = qr * W
                    # ---- input slices for this quarter (per-di DMAs for
                    # fine-grained matmul start) ----
                    xin = xinp.tile([P, DT, 512], BF16, tag="xin")
                    for di in range(DT):
                        nc.sync.dma_start(xin[:, di, :], xkvT_t[:, di, s0 : s0 + 512])
                    xq = xqp.tile([P, DT, W], BF16, tag="xq")
                    nc.sync.dma_start(xq, xqT_t[:, :, q0 : q0 + W])
                    # broadcast global q indices for this strip to all
                    # partitions (for the causal mask)
                    qgrid = sm.tile([P, W], F32, tag="qgrid")
                    qg_sl = qg[q0 : q0 + W]
                    nc.gpsimd.dma_start(
                        qgrid,
                        bass.AP(
                            tensor=qg_sl.tensor,
                            offset=qg_sl.offset,
                            ap=[[0, P]] + list(qg_sl.ap),
                        ),
                    )

                    # ---- K^T half: out[d_out_half, k] accumulated over d_in;
                    # stage -> DRAM -> pairwise AllGather -> full kT quarter ----
                    ksg = stgp.tile([P, HDT, 512], BF16, tag="ksg")
                    for do in range(HDT):
                        ps = psA.tile([P, 512], F32, tag="psA")
                        for di in range(DT):
                            nc.tensor.matmul(
                                ps,
                                wk[:, di, do * P : (do + 1) * P],
                                xin[:, di, :],
                                start=(di == 0),
                                stop=(di == DT - 1),
                            )
                        nc.vector.tensor_copy(ksg[:, do, :], ps)
                    kin_d = dram.tile([HD, 512], BF16, tag="kin")
                    nc.sync.dma_start(
                        kin_d.rearrange("(a p) s -> p a s", p=P), ksg
                    )
                    kout_d = dram.tile([D, 512], BF16, tag="kout")
                    nc.gpsimd.collective_compute(
                        "AllGather",
                        mybir.AluOpType.bypass,
                        replica_groups=groups,
                        ins=[kin_d.opt()],
                        outs=[kout_d.opt()],
                    )
                    nc.sync.dma_start(
                        kT[:, :, s0 : s0 + 512],
                        kout_d.rearrange("(a p) s -> p a s", p=P),
                    )

                    # ---- V half: out[k, d_out_half] accumulated over d_in;
                    # same staged exchange ----
                    vsg = stgp.tile([P, 4, HD], BF16, tag="vsg")
                    for st in range(4):
                        ps = psA.tile([P, 512], F32, tag="psA")
                        for di in range(DT):
                            nc.tensor.matmul(
                                ps,
                                xin[:, di, st * P : (st + 1) * P],
                                wv[:, di, :],
                                start=(di == 0),
                                stop=(di == DT - 1),
                            )
                        nc.vector.tensor_copy(vsg[:, st, :], ps)
                    vin_d = dram.tile([512, HD], BF16, tag="vin")
                    nc.sync.dma_start(
                        vin_d.rearrange("(a p) o -> p a o", p=P), vsg
                    )
                    vout_d = dram.tile([2 * 512, HD], BF16, tag="vout")
                    nc.gpsimd.collective_compute(
                        "AllGather",
                        mybir.AluOpType.bypass,
                        replica_groups=groups,
                        ins=[vin_d.opt()],
                        outs=[vout_d.opt()],
                    )
                    vout_t = vout_d.rearrange("(dh a p) o -> dh p a o", dh=2, p=P)
                    for dh in range(2):
                        nc.sync.dma_start(
                            vT[:, qr * 4 : (qr + 1) * 4, dh * HD : (dh + 1) * HD],
                            vout_t[dh],
                        )

                    # ---- Q^T strip: out[d_out, q] accumulated over d_in ----
                    for do in range(DT):
                        ps = psA.tile([P, W], F32, tag="psA")
                        for di in range(DT):
                            nc.tensor.matmul(
                                ps,
                                wq[:, di, do * P : (do + 1) * P],
                                xq[:, di, :],
                                start=(di == 0),
                                stop=(di == DT - 1),
                            )
                        nc.vector.tensor_copy(qT[:, do, q0 : q0 + W], ps)

                    # ---- attention strip qr: S^T -> exp -> mask -> P^T ----
                    # This strip holds global q-tiles 4qr+h and 4qr+2+h, so
                    # k-tiles >= ext_kt are fully masked and skipped.
                    ext_kt = 4 * (qr + 1)
                    pT = ptp.tile([P, KT, W], BF16, tag="pT")
                    for kt in range(ext_kt):
                        ps = psA.tile([P, W], F32, tag="psA")
                        for di in range(DT):
                            nc.tensor.matmul(
                                ps,
                                kT[:, di, kt * P : (kt + 1) * P],
                                qT[:, di, q0 : q0 + W],
                                start=(di == 0),
                                stop=(di == DT - 1),
                            )
                        et = sm.tile([P, W], F32, tag="et")
                        nc.scalar.activation(
                            et, ps, mybir.ActivationFunctionType.Exp, scale=float(SCALE)
                        )
                        mt = sm.tile([P, W], F32, tag="mt")
                        nc.vector.tensor_scalar(
                            mt,
                            qgrid,
                            kvecf[:, kt : kt + 1],
                            None,
                            op0=mybir.AluOpType.is_ge,
                        )
                        nc.vector.tensor_mul(pT[:, kt, :], et, mt)

                    # ---- context = P^T.T @ V (V resident in SBUF) ----
                    ncq = W // P
                    cps = [
                        pscp.tile([P, D], F32, tag="psc", name=f"cps{i}")
                        for i in range(ncq)
                    ]
                    lps = [
                        pslp.tile([P, 2], F32, tag="psl", name=f"lps{i}")
                        for i in range(ncq)
                    ]
                    for kt in range(ext_kt):
                        for qt in range(ncq):
                            ej = 2 * (qr * ncq + qt) + 2  # this position's extent
                            if kt >= ej:
                                continue
                            lhs = pT[:, kt, qt * P : (qt + 1) * P]
                            nc.tensor.matmul(
                                cps[qt][:, 0:512],
                                lhs,
                                vT[:, kt, 0:512],
                                start=(kt == 0),
                                stop=(kt == ej - 1),
                            )
                            nc.tensor.matmul(
                                cps[qt][:, 512:1024],
                                lhs,
                                vT[:, kt, 512:1024],
                                start=(kt == 0),
                                stop=(kt == ej - 1),
                            )
                            nc.tensor.matmul(
                                lps[qt],
                                lhs,
                                ones,
                                start=(kt == 0),
                                stop=(kt == ej - 1),
                            )
                    for qt in range(ncq):
                        qrow = q0 + qt * P
                        rt = sm.tile([P, 1], F32, tag="rt")
                        nc.vector.reciprocal(rt, lps[qt][:, 0:1])
                        ot = outp.tile([P, D], F32, tag="ot")
                        nc.vector.tensor_scalar_mul(ot, cps[qt], rt)
                        nc.sync.dma_start(out_d[qrow : qrow + P, :], ot)
    nc.compile()
    return nc


def _get_nc(key=8):
    if key not in _NC_CACHE:
        _NC_CACHE[key] = build_nc(n_cores=key if isinstance(key, int) else 8)
    return _NC_CACHE[key]


def _qsel(h):
    """Query rows for core-half h: global q-tiles h, 2+h, ..., 14+h.

    Position p's tile 2p+h needs only k < (2p+h+1)*128, letting the kernel
    skip fully-masked k-tiles at compile time with a core-uniform program."""
    tiles = np.arange(8) * 2 + h
    return (tiles[:, None] * P + np.arange(P)[None, :]).reshape(-1)


def make_in_maps(x, Wq, Wk, Wv, n_cores=8):
    x = np.asarray(x, dtype=np.float32)
    Wq = np.ascontiguousarray(np.asarray(Wq, dtype=np.float32)).astype(BF_NP)
    Wk = np.ascontiguousarray(np.asarray(Wk, dtype=np.float32)).astype(BF_NP)
    Wv = np.ascontiguousarray(np.asarray(Wv, dtype=np.float32)).astype(BF_NP)
    in_maps = []
    for c in range(n_cores):
        b, h = c // 2, c % 2
        qsel = _qsel(h)
        xbT = np.ascontiguousarray(x[b].T).astype(BF_NP)
        in_maps.append(
            {
                "xkvT": xbT,
                "xqT": np.ascontiguousarray(xbT[:, qsel]),
                "qg": qsel.astype(np.float32),
                "Wq": Wq,
                "Wk": np.ascontiguousarray(Wk[:, h * HD : (h + 1) * HD]),
                "Wv": np.ascontiguousarray(Wv[:, h * HD : (h + 1) * HD]),
            }
        )
    return in_maps


def kernel(x, Wq, Wk, Wv, _trace=False, _nc_key=8):
    nc = _get_nc(8)
    in_maps = make_in_maps(x, Wq, Wk, Wv)
    res = run_bass_kernel_spmd(nc, in_maps, core_ids=list(range(8)), trace=_trace)
    out = np.empty((B, S, D), dtype=np.float32)
    for c in range(8):
        b, h = c // 2, c % 2
        out[b, _qsel(h), :] = res.results[c]["out"]
    if _trace:
        kernel.last_results = res
    return out


# revision 8
# speedup vs baseline: 1.4888x; 1.0619x over previous
"""Trainium2 Bass kernel: single-head causal self-attention (fused streaming,
pair-split K/V projection with a software-pipelined AllGather exchange).

Reference computation (per batch b):
    Q = x @ Wq ; K = x @ Wk ; V = x @ Wv          (x: [S, D])
    S_sc = Q @ K^T / sqrt(D), causal masked
    out  = softmax(S_sc) @ V

Sharding: 8 cores, 4 batches -> core c handles batch b = c//2 and the
interleaved query half h = c%2 (q-tiles 2p+h, 1024 query rows). The K/V
projections for batch b are split across the pair by output column: core h
computes K^T/V for d_out columns [h*512, (h+1)*512) only (its Wk/Wv input
is the corresponding half of the weight matrix), and the halves are
exchanged with one combined pairwise AllGather per key-quarter through
DRAM bounce buffers. This halves the projection FLOPs vs computing K/V
redundantly.

Pipelined streaming structure: quarter qr projects its K^T/V half,
launches the AllGather, projects this core's Q^T strip, then runs the
attention strip for quarter qr-1 (whose gather already landed). The
attention strip for the last quarter runs after the loop. A strip's
causal extent is exactly the quarters processed up to it, so no masked
k-tile work is wasted beyond the 128-row tile granularity. K^T, V and
Q^T all stay resident in SBUF in bf16.

Scores are computed transposed (S^T[k, q]) so the softmax k-reduction
lands on partitions; row sums ride an extra ones-column matmul on the
same stationary P^T tiles. No max-subtraction (scores ~ N(0,1); exp is
safe in fp32). Causal mask built on device from a global-q-index input.
"""

import sys

try:
    import concourse.bass as bass  # noqa: F401
except ImportError:
    sys.path.insert(0, "/opt/trn_rl_repo")

import ml_dtypes
import numpy as np

import concourse.bass as bass
import concourse.tile as tile
from concourse import bacc, mybir
from concourse.bass_utils import run_bass_kernel_spmd

B, S, D = 4, 2048, 1024
NQ = 1024  # query rows per core
NK = 2048  # keys per core
P = 128
DT = D // P  # 8 d tiles
KT = NK // P  # 16 k tiles
W = 256  # queries per quarter-strip
NQR = 4  # key quarters (512 keys each)
HD = D // 2  # per-core K/V projection half (d_out columns)
HDT = DT // 2  # 4 d_out tiles per half
F32 = mybir.dt.float32
BF16 = mybir.dt.bfloat16
SCALE = 1.0 / np.sqrt(np.float32(D))  # 0.03125
BF_NP = ml_dtypes.bfloat16

_NC_CACHE = {}


def build_nc(n_cores=8):
    groups = [[2 * i, 2 * i + 1] for i in range(n_cores // 2)]
    nc = bacc.Bacc(None, target_bir_lowering=False, num_devices=n_cores)
    xkvT = nc.dram_tensor("xkvT", [D, NK], BF16, kind="ExternalInput")
    xqT = nc.dram_tensor("xqT", [D, NQ], BF16, kind="ExternalInput")
    qg = nc.dram_tensor("qg", [NQ], F32, kind="ExternalInput")
    wq_d = nc.dram_tensor("Wq", [D, D], BF16, kind="ExternalInput")
    wk_d = nc.dram_tensor("Wk", [D, HD], BF16, kind="ExternalInput")
    wv_d = nc.dram_tensor("Wv", [D, HD], BF16, kind="ExternalInput")
    out_d = nc.dram_tensor("out", [NQ, D], F32, kind="ExternalOutput")

    with tile.TileContext(nc) as tc:
        with (
            tc.tile_pool(name="persist", bufs=1) as persist,
            tc.tile_pool(name="misc", bufs=1) as misc,
        ):
            # Persistent SBUF residents (bf16): K^T [d, NK], V [k, D] per
            # k-tile, Q^T [d, NQ]
            kT = persist.tile([P, DT, NK], BF16, tag="kT")
            vT = persist.tile([P, KT, D], BF16, tag="vT")
            qT = persist.tile([P, DT, NQ], BF16, tag="qT")
            wk = persist.tile([P, DT, HD], BF16, tag="wk")
            wv = persist.tile([P, DT, HD], BF16, tag="wv")
            wq = persist.tile([P, DT, D], BF16, tag="wq")

            # Small constants: ones columns (moving operand of the row-sum
            # matmul), k-index vectors for the causal mask
            ones_f = misc.tile([P, 2], F32, tag="ones_f")
            nc.vector.memset(ones_f, 1.0)
            ones = misc.tile([P, 2], BF16, tag="ones")
            nc.vector.tensor_copy(ones, ones_f)
            pvec_i = misc.tile([P, 1], mybir.dt.int32, tag="pvec_i")
            nc.gpsimd.iota(pvec_i, pattern=[[0, 1]], base=0, channel_multiplier=1)
            pvec = misc.tile([P, 1], F32, tag="pvec")
            nc.vector.tensor_copy(pvec, pvec_i)
            kvecf = misc.tile([P, KT], F32, tag="kvecf")
            for kt in range(KT):
                nc.vector.tensor_scalar_add(kvecf[:, kt : kt + 1], pvec, float(kt * P))

            # Weight DMAs (scalar queue; xin/xq ride the sync queue in
            # parallel). wk first: quarter 0's K matmuls gate PE start.
            wk_t = wk_d.rearrange("(a p) o -> p a o", p=P)
            wv_t = wv_d.rearrange("(a p) o -> p a o", p=P)
            wq_t = wq_d.rearrange("(a p) o -> p a o", p=P)
            for di in range(DT):
                nc.scalar.dma_start(wk[:, di, :], wk_t[:, di, :])
            for di in range(DT):
                nc.scalar.dma_start(wv[:, di, :], wv_t[:, di, :])
            for di in range(DT):
                nc.scalar.dma_start(wq[:, di, :], wq_t[:, di, :])

            xkvT_t = xkvT.rearrange("(a p) s -> p a s", p=P)
            xqT_t = xqT.rearrange("(a p) s -> p a s", p=P)

            with (
                tc.tile_pool(name="xin", bufs=2) as xinp,
                tc.tile_pool(name="xq", bufs=2) as xqp,
                tc.tile_pool(name="stg", bufs=2) as stgp,
                tc.tile_pool(name="dram", bufs=2, space="DRAM") as dram,
                tc.tile_pool(name="sm", bufs=4) as sm,
                tc.tile_pool(name="pt", bufs=1) as ptp,
                tc.tile_pool(name="outp", bufs=2) as outp,
                tc.tile_pool(name="psA", bufs=2, space="PSUM") as psA,
                tc.tile_pool(name="psc", bufs=2, space="PSUM") as pscp,
                tc.tile_pool(name="psl", bufs=2, space="PSUM") as pslp,
            ):
                qgrids = {}

                def attn(qs):
                    """Attention strip qs: S^T -> exp -> mask -> P^T -> @V."""
                    q0 = qs * W
                    qgrid = qgrids.pop(qs)
                    # This strip holds global q-tiles 4qs+h and 4qs+2+h, so
                    # k-tiles >= ext_kt are fully masked and skipped.
                    ext_kt = 4 * (qs + 1)
                    pT = ptp.tile([P, KT, W], BF16, tag="pT", name="pT")
                    for kt in range(ext_kt):
                        ps = psA.tile([P, W], F32, tag="psA", name="ps")
                        for di in range(DT):
                            nc.tensor.matmul(
                                ps,
                                kT[:, di, kt * P : (kt + 1) * P],
                                qT[:, di, q0 : q0 + W],
                                start=(di == 0),
                                stop=(di == DT - 1),
                            )
                        et = sm.tile([P, W], F32, tag="et", name="et")
                        nc.scalar.activation(
                            et, ps, mybir.ActivationFunctionType.Exp, scale=float(SCALE)
                        )
                        mt = sm.tile([P, W], F32, tag="mt", name="mt")
                        nc.vector.tensor_scalar(
                            mt,
                            qgrid,
                            kvecf[:, kt : kt + 1],
                            None,
                            op0=mybir.AluOpType.is_ge,
                        )
                        nc.vector.tensor_mul(pT[:, kt, :], et, mt)

                    # context = P^T.T @ V (V resident in SBUF); row sums l
                    # ride a ones-column matmul on the same stationary P^T
                    ncq = W // P
                    cps = [
                        pscp.tile([P, D], F32, tag="psc", name=f"cps{i}")
                        for i in range(ncq)
                    ]
                    lps = [
                        pslp.tile([P, 2], F32, tag="psl", name=f"lps{i}")
                        for i in range(ncq)
                    ]
                    for kt in range(ext_kt):
                        for qt in range(ncq):
                            ej = 2 * (qs * ncq + qt) + 2  # this position's extent
                            if kt >= ej:
                                continue
                            lhs = pT[:, kt, qt * P : (qt + 1) * P]
                            nc.tensor.matmul(
                                cps[qt][:, 0:512],
                                lhs,
                                vT[:, kt, 0:512],
                                start=(kt == 0),
                                stop=(kt == ej - 1),
                            )
                            nc.tensor.matmul(
                                cps[qt][:, 512:1024],
                                lhs,
                                vT[:, kt, 512:1024],
                                start=(kt == 0),
                                stop=(kt == ej - 1),
                            )
                            nc.tensor.matmul(
                                lps[qt],
                                lhs,
                                ones,
                                start=(kt == 0),
                                stop=(kt == ej - 1),
                            )
                    for qt in range(ncq):
                        qrow = q0 + qt * P
                        rt = sm.tile([P, 1], F32, tag="rt", name="rt")
                        nc.vector.reciprocal(rt, lps[qt][:, 0:1])
                        ot = outp.tile([P, D], F32, tag="ot", name="ot")
                        nc.vector.tensor_scalar_mul(ot, cps[qt], rt)
                        nc.sync.dma_start(out_d[qrow : qrow + P, :], ot)

                for qr in range(NQR):
                    s0 = qr * 512
                    q0 = qr * W
                    # ---- input slices for this quarter (per-di DMAs for
                    # fine-grained matmul start) ----
                    xin = xinp.tile([P, DT, 512], BF16, tag="xin")
                    for di in range(DT):
                        nc.sync.dma_start(xin[:, di, :], xkvT_t[:, di, s0 : s0 + 512])
                    xq = xqp.tile([P, DT, W], BF16, tag="xq")
                    nc.sync.dma_start(xq, xqT_t[:, :, q0 : q0 + W])
                    # broadcast global q indices for this strip to all
                    # partitions (for the causal mask)
                    qgrid = sm.tile([P, W], F32, tag="qgrid")
                    qg_sl = qg[q0 : q0 + W]
                    nc.gpsimd.dma_start(
                        qgrid,
                        bass.AP(
                            tensor=qg_sl.tensor,
                            offset=qg_sl.offset,
                            ap=[[0, P]] + list(qg_sl.ap),
                        ),
                    )
                    qgrids[qr] = qgrid

                    # ---- K^T half: out[d_out_half, k] accumulated over d_in ----
                    kvin_d = dram.tile([2 * HD, 512], BF16, tag="kvin")
                    ksg = stgp.tile([P, HDT, 512], BF16, tag="ksg")
                    for do in range(HDT):
                        ps = psA.tile([P, 512], F32, tag="psA")
                        for di in range(DT):
                            nc.tensor.matmul(
                                ps,
                                wk[:, di, do * P : (do + 1) * P],
                                xin[:, di, :],
                                start=(di == 0),
                                stop=(di == DT - 1),
                            )
                        nc.vector.tensor_copy(ksg[:, do, :], ps)
                    nc.sync.dma_start(
                        kvin_d[0:HD].rearrange("(a p) s -> p a s", p=P), ksg
                    )

                    # ---- V half: out[k, d_out_half] accumulated over d_in ----
                    vsg = stgp.tile([P, 4, HD], BF16, tag="vsg")
                    for st in range(4):
                        ps = psA.tile([P, 512], F32, tag="psA")
                        for di in range(DT):
                            nc.tensor.matmul(
                                ps,
                                xin[:, di, st * P : (st + 1) * P],
                                wv[:, di, :],
                                start=(di == 0),
                                stop=(di == DT - 1),
                            )
                        nc.vector.tensor_copy(vsg[:, st, :], ps)
                    nc.sync.dma_start(
                        kvin_d[HD : 2 * HD].rearrange("(a p) o -> p a o", p=P), vsg
                    )

                    # ---- combined pairwise AllGather: [myK|myV] x2 ranks ----
                    kvout_d = dram.tile([4 * HD, 512], BF16, tag="kvout")
                    nc.gpsimd.collective_compute(
                        "AllGather",
                        mybir.AluOpType.bypass,
                        replica_groups=groups,
                        ins=[kvin_d.opt()],
                        outs=[kvout_d.opt()],
                    )
                    # readback: rank r's K half at rows [2r*HD, (2r+1)*HD)
                    kv_t = kvout_d.rearrange("(r b a p) s -> r b p a s", r=2, b=2, p=P)
                    for r in range(2):
                        nc.sync.dma_start(
                            kT[:, r * HDT : (r + 1) * HDT, s0 : s0 + 512], kv_t[r, 0]
                        )
                        nc.sync.dma_start(
                            vT[:, qr * 4 : (qr + 1) * 4, r * HD : (r + 1) * HD],
                            kv_t[r, 1],
                        )

                    # ---- Q^T strip: out[d_out, q] accumulated over d_in ----
                    for do in range(DT):
                        ps = psA.tile([P, W], F32, tag="psA")
                        for di in range(DT):
                            nc.tensor.matmul(
                                ps,
                                wq[:, di, do * P : (do + 1) * P],
                                xq[:, di, :],
                                start=(di == 0),
                                stop=(di == DT - 1),
                            )
                        nc.vector.tensor_copy(qT[:, do, q0 : q0 + W], ps)

                    # ---- attention strip qr-1 (its gather has landed) ----
                    if qr >= 1:
                        attn(qr - 1)
                attn(NQR - 1)
    nc.compile()
    return nc


def _get_nc(key=8):
    if key not in _NC_CACHE:
        _NC_CACHE[key] = build_nc(n_cores=key if isinstance(key, int) else 8)
    return _NC_CACHE[key]


def _qsel(h):
    """Query rows for core-half h: global q-tiles h, 2+h, ..., 14+h.

    Position p's tile 2p+h needs only k < (2p+h+1)*128, letting the kernel
    skip fully-masked k-tiles at compile time with a core-uniform program."""
    tiles = np.arange(8) * 2 + h
    return (tiles[:, None] * P + np.arange(P)[None, :]).reshape(-1)


def make_in_maps(x, Wq, Wk, Wv, n_cores=8):
    x = np.asarray(x, dtype=np.float32)
    Wq = np.ascontiguousarray(np.asarray(Wq, dtype=np.float32)).astype(BF_NP)
    Wk = np.ascontiguousarray(np.asarray(Wk, dtype=np.float32)).astype(BF_NP)
    Wv = np.ascontiguousarray(np.asarray(Wv, dtype=np.float32)).astype(BF_NP)
    in_maps = []
    for c in range(n_cores):
        b, h = c // 2, c % 2
        qsel = _qsel(h)
        xbT = np.ascontiguousarray(x[b].T).astype(BF_NP)
        in_maps.append(
            {
                "xkvT": xbT,
                "xqT": np.ascontiguousarray(xbT[:, qsel]),
                "qg": qsel.astype(np.float32),
                "Wq": Wq,
                "Wk": np.ascontiguousarray(Wk[:, h * HD : (h + 1) * HD]),
                "Wv": np.ascontiguousarray(Wv[:, h * HD : (h + 1) * HD]),
            }
        )
    return in_maps


def kernel(x, Wq, Wk, Wv, _trace=False, _nc_key=8):
    nc = _get_nc(8)
    in_maps = make_in_maps(x, Wq, Wk, Wv)
    res = run_bass_kernel_spmd(nc, in_maps, core_ids=list(range(8)), trace=_trace)
    out = np.empty((B, S, D), dtype=np.float32)
    for c in range(8):
        b, h = c // 2, c % 2
        out[b, _qsel(h), :] = res.results[c]["out"]
    if _trace:
        kernel.last_results = res
    return out
